# revision 9
# baseline (speedup 1.0000x reference)
"""Trainium2 Bass kernel for windowed (local) causal self-attention.

Reference computation (per batch element, fp32):
    q = x @ Wq.T + bq ; k = x @ Wk.T + bk ; v = x @ Wv.T + bv
    per non-overlapping window of 256 tokens:
        attn = softmax(causal_mask(q k^T * HEAD_DIM**-0.5))
        out  = attn @ v
    o = out @ Wo.T + bo + x

Algebraic reduction (the reference has no head split, so scores contract
over the full E=1024):
    q_i.k_j = x_i^T (Wq^T Wk) x_j + x_i.(Wq^T bk) + (Wk^T bq).x_j + bq.bk
The second and fourth terms are constant along the softmax axis and
cancel; with M = Wq^T Wk and vvec = Wk^T bq:
    scores = (X M + 1 vvec^T) X^T      (one projection instead of two)
Since softmax rows sum to 1,
    attn @ (X Wv^T + 1 bv^T) @ Wo^T + 1 bo^T = attn @ (X P^T) + 1 b'^T
with P = Wo Wv and b' = Wo bv + bo (one projection instead of two).
b' is folded into the residual copy of x on the host.  Device work per
token is therefore 2 E^2 MACs of projection + windowed attention, ~55%
of the naive PE work.

Sharding: data-parallel over (batch, window): 64 window-blocks of 256
tokens -> 8 cores x 8 windows.  M/P replicated (8MB instead of 16MB).

Per-core kernel strategy (all matmuls float32r: 1 cycle/row at N>=256):
  - M, P resident in SBUF; x streamed as host-transposed xT [E, t] plus
    a token-major residual copy (x + b').
  - windows processed in groups [(0,),(1,),(2,3),(4,5),(6,7)]: the two
    solo groups let compute start after only xT[w0] (1MB) + M half
    (2MB) of DMA; the pairs run the y-projection at N=512 to halve the
    PE stationary-load count.
  - per window: scores = yT^T-blocks @ xT accumulated over 8 e-tiles
    (vvec bias pre-fused into yT via the ACT psum evac); causal mask
    added during psum evac; scale+exp+row-sum fused in one ACT op
    (accum_out); attn normalized by 1/sum on DVE and PE-transposed to
    attnT; Z = X P^T token-major (ACT-copy evac to keep DVE off the
    critical path); out = attnT^T @ Z directly token-major, residual
    (x + b') fused into the DVE evac; stores issued from the DVE queue
    so they never queue behind prefetch DMAs and drain immediately.
  - weight DMA (8MB) is chunked and interleaved with the first two
    solo windows' emission, paced with tiny PE warmup transposes.
"""
import sys

sys.path.insert(0, "/opt/trn_rl_repo")

import numpy as np

import concourse.bass as bass
import concourse.bacc as bacc
import concourse.mybir as mybir
import concourse.tile as tile
from concourse.bass_utils import run_bass_kernel_spmd

F32 = mybir.dt.float32
F32R = mybir.dt.float32r
F16 = mybir.dt.float16
AF = mybir.ActivationFunctionType

E = 1024          # embed dim
ET = E // 128     # e-tiles
W = 256           # window size
NW = 8            # windows per core
T = NW * W        # tokens per core
N_CORES = 8
SCALE = (E // 16) ** (-0.5)  # HEAD_DIM ** -0.5 = 0.125
NEG = -1.0e30

GROUPS = [(0,), (1,), (2, 3), (4, 5), (6, 7)]


def build_nc(nw=NW):
    t_core = nw * W
    nc = bacc.Bacc("TRN2", target_bir_lowering=False, debug=False)

    # x: token-major residual copy with b' = Wo@bv + bo pre-added (host)
    x_d = nc.dram_tensor("x", [t_core, E], F16, kind="ExternalInput")
    xt_d = nc.dram_tensor("xt", [E, t_core], F16, kind="ExternalInput")
    m_d = nc.dram_tensor("wm", [E, E], F16, kind="ExternalInput")   # Wq^T @ Wk
    p_d = nc.dram_tensor("wz", [E, E], F16, kind="ExternalInput")   # (Wo @ Wv)^T
    vv_d = nc.dram_tensor("vv", [128, ET], F32, kind="ExternalInput")  # Wk^T @ bq
    o_d = nc.dram_tensor("o", [t_core, E], F16, kind="ExternalOutput")

    # host-side constants baked into the NEFF
    mask_np = np.zeros((2, 128, W), dtype=np.float32)
    for qt in range(2):
        r = np.arange(128)[:, None] + qt * 128
        c = np.arange(W)[None, :]
        mask_np[qt][c > r] = NEG
    mask_d = nc.inline_tensor(mask_np, "mask")
    ident_d = nc.inline_tensor(np.eye(128, dtype=np.float16), "ident")

    with tile.TileContext(nc) as tc:
        with (
            tc.tile_pool(name="wgt", bufs=1) as wp,
            tc.tile_pool(name="cp", bufs=1) as cp,
            tc.tile_pool(name="xp", bufs=4) as xp,
            tc.tile_pool(name="xtp", bufs=2) as xtp,
            tc.tile_pool(name="ytp", bufs=2) as ytp,
            tc.tile_pool(name="zp", bufs=2) as zp,
            tc.tile_pool(name="sp", bufs=2) as sp,
            tc.tile_pool(name="ap_", bufs=2) as apool,
            tc.tile_pool(name="atp", bufs=2) as atp,
            tc.tile_pool(name="smp", bufs=4) as smp,
            tc.tile_pool(name="op", bufs=3) as op,
            tc.tile_pool(name="ps_y", bufs=2, space=bass.MemorySpace.PSUM) as ps_y,
            tc.tile_pool(name="ps_z", bufs=2, space=bass.MemorySpace.PSUM) as ps_z,
            tc.tile_pool(name="ps_o", bufs=3, space=bass.MemorySpace.PSUM) as ps_o,
            tc.tile_pool(name="ps_tr", bufs=1, space=bass.MemorySpace.PSUM) as ps_tr,
        ):
            # ---- resident constants (gpsimd queue; tiny) ----
            ident = cp.tile([128, 128], F16, tag="ident")
            nc.gpsimd.dma_start(ident[:], ident_d.ap())
            masks = cp.tile([128, 2, W], F32, tag="mask")
            for qt in range(2):
                nc.gpsimd.dma_start(masks[:, qt, :], mask_d.ap()[qt])
            vv_sb = cp.tile([128, ET], F32, tag="vv")
            nc.gpsimd.dma_start(vv_sb[:], vv_d.ap())

            # ---- resident weights: wsb[m][p, ei, eo] = Wm[ei*128+p, eo] ----
            wsb = {
                "m": wp.tile([128, ET, E], F16, tag="wm", name="wmsb"),
                "z": wp.tile([128, ET, E], F16, tag="wz", name="wzsb"),
            }
            w_d = {"m": m_d, "z": p_d}

            def load_weight(m, half, warm=False):
                # one 3D DMA per (half, ei-quadrant): 1MB transfers with 2KB
                # contiguous rows keep the DMA engines at full rate
                wr = w_d[m].ap().rearrange("(a p) n -> a p n", p=128)
                for eq in range(0, ET, 4):
                    nc.sync.dma_start(
                        wsb[m][:, eq : eq + 4, half * 512 : (half + 1) * 512],
                        wr[eq : eq + 4, :, half * 512 : (half + 1) * 512].transpose(
                            [1, 0, 2]
                        ),
                    )
                    if warm:
                        # keep the PE activity monitor warm through the
                        # DMA-bound phase: a tiny transpose per arriving
                        # chunk, paced by the DMA itself
                        wps = ps_tr.tile([128, 128], F16, tag="tr", name="warm")
                        nc.tensor.transpose(
                            wps[:],
                            wsb[m][:, eq, half * 512 : half * 512 + 128],
                            ident[:],
                        )

            xtr = xt_d.ap().rearrange("(a p) t -> a p t", p=128)
            xT_tiles = {}

            def load_xT(gi):
                # group tile is always pair-sized; solo groups fill half.
                # one DMA per window so solo-group compute can begin after
                # a single 1MB transfer.
                g = GROUPS[gi]
                t = xtp.tile([128, ET, 2 * W], F16, tag="xT")
                for wi, w in enumerate(g):
                    nc.sync.dma_start(
                        t[:, :, wi * W : (wi + 1) * W],
                        xtr[:, :, w * W : (w + 1) * W].transpose([1, 0, 2]),
                    )
                xT_tiles[gi] = t

            for gi, g in enumerate(GROUPS):
                pw = len(g) * W

                if gi == 0:
                    load_weight("m", half=0, warm=True)
                    load_xT(0)
                    load_weight("m", half=1, warm=True)
                    load_xT(1)  # prefetch window 1 behind M
                elif gi + 1 < len(GROUPS):
                    load_xT(gi + 1)
                xT = xT_tiles[gi]

                # ---- y projection -> [e_out, t] layout, vvec bias fused ----
                yT = ytp.tile([128, ET, 2 * W], F16, tag="yT")
                for eo in range(ET):
                    if gi == 0 and eo == 4:
                        load_weight("z", half=0, warm=True)
                    pp = ps_y.tile([128, 2 * W], F32, tag="y")
                    for ei in range(ET):
                        nc.tensor.matmul(
                            pp[:, :pw],
                            wsb["m"][:, ei, eo * 128 : (eo + 1) * 128],
                            xT[:, ei, :pw],
                            start=(ei == 0),
                            stop=(ei == ET - 1),
                        )
                    if eo % 2 == 0:
                        nc.scalar.add(
                            yT[:, eo, :pw], pp[:, :pw], vv_sb[:, eo : eo + 1]
                        )
                    else:
                        nc.vector.tensor_scalar_add(
                            yT[:, eo, :pw], pp[:, :pw], vv_sb[:, eo : eo + 1]
                        )

                if gi == 0:
                    load_weight("z", half=1, warm=True)

                for wi, w in enumerate(g):
                    tok0 = w * W
                    two0 = wi * W  # token offset inside the group tiles

                    # residual (x + b'), token-major; needed only at out evac
                    x_w = []
                    for tt in range(2):
                        xt_ = xp.tile([128, E], F16, tag="x")
                        nc.sync.dma_start(
                            xt_[:],
                            x_d.ap()[tok0 + tt * 128 : tok0 + (tt + 1) * 128, :],
                        )
                        x_w.append(xt_)

                    # ---- scores + softmax + transpose(attn) ----
                    aT = []
                    for ktt in range(2):
                        t_ = atp.tile([128, W], F16, tag="aT", name=f"aT{ktt}")
                        aT.append(t_)
                    for qt in range(2):
                        # causal: query rows [0,128) attend only k < 128, so
                        # the qt0 score block is [128,128] and its upper
                        # attn block (ktt=1) is exactly zero -> skipped.
                        kw = 128 if qt == 0 else W
                        sc = ps_y.tile([128, 2 * W], F32, tag="y")
                        for ei in range(ET):
                            nc.tensor.matmul(
                                sc[:, :kw],
                                yT[:, ei, two0 + qt * 128 : two0 + (qt + 1) * 128],
                                xT[:, ei, two0 : two0 + kw],
                                start=(ei == 0),
                                stop=(ei == ET - 1),
                            )
                        s_sb = sp.tile([128, W], F32, tag="s")
                        nc.vector.tensor_add(s_sb[:, :kw], sc[:, :kw], masks[:, qt, :kw])
                        sums = smp.tile([128, 1], F32, tag="sum")
                        nc.scalar.activation(
                            s_sb[:, :kw], s_sb[:, :kw], AF.Exp, scale=SCALE,
                            accum_out=sums[:],
                        )
                        rec = smp.tile([128, 1], F32, tag="rec")
                        nc.vector.reciprocal(rec[:], sums[:])
                        a_sb = apool.tile([128, W], F16, tag="a")
                        nc.vector.tensor_scalar_mul(a_sb[:, :kw], s_sb[:, :kw], rec[:])
                        for ktt in range(kw // 128):
                            ptr = ps_tr.tile([128, 128], F16, tag="tr", name="ptra")
                            nc.tensor.transpose(
                                ptr[:], a_sb[:, ktt * 128 : (ktt + 1) * 128], ident[:]
                            )
                            nc.vector.tensor_copy(
                                aT[ktt][:, qt * 128 : (qt + 1) * 128], ptr[:]
                            )

                    # ---- Z projection (token-major): Z = X P^T ----
                    z_w = [
                        zp.tile([128, E], F16, tag="z", name=f"z{tt}")
                        for tt in range(2)
                    ]
                    for eoh in range(2):
                        for tt in range(2):
                            pv = ps_z.tile([128, 512], F32, tag="z")
                            for ei in range(ET):
                                nc.tensor.matmul(
                                    pv[:],
                                    xT[:, ei, two0 + tt * 128 : two0 + (tt + 1) * 128],
                                    wsb["z"][:, ei, eoh * 512 : (eoh + 1) * 512],
                                    start=(ei == 0),
                                    stop=(ei == ET - 1),
                                )
                            nc.scalar.copy(
                                z_w[tt][:, eoh * 512 : (eoh + 1) * 512], pv[:]
                            )

                    # ---- out = attn @ Z directly token-major + residual ----
                    for qt in range(2):
                        nk = 1 if qt == 0 else 2  # qt0 rows only see ktt=0
                        for eoh in range(2):
                            po = ps_o.tile([128, 512], F32, tag="o")
                            for ktt in range(nk):
                                nc.tensor.matmul(
                                    po[:],
                                    aT[ktt][:, qt * 128 : (qt + 1) * 128],
                                    z_w[ktt][:, eoh * 512 : (eoh + 1) * 512],
                                    start=(ktt == 0),
                                    stop=(ktt == nk - 1),
                                )
                            o_sb = op.tile([128, 512], F16, tag="o")
                            if w == nw - 1 and eoh == 1:
                                # tail: drain the last window's two eoh=1
                                # chunks via ACT+GpSimd so they run in
                                # parallel with the DVE chunks
                                o_t = op.tile([128, 512], F32, tag="ot")
                                nc.scalar.copy(o_t[:], po[:])
                                nc.gpsimd.tensor_add(
                                    o_sb[:],
                                    o_t[:],
                                    x_w[qt][:, eoh * 512 : (eoh + 1) * 512],
                                )
                            else:
                                nc.vector.tensor_add(
                                    o_sb[:],
                                    po[:],
                                    x_w[qt][:, eoh * 512 : (eoh + 1) * 512],
                                )
                            # stores on the sync/gpsimd queues (idle in
                            # steady state): the scalar queue's ACT ops sit
                            # on the softmax critical path, keep it clear
                            q = nc.sync if eoh == 0 else nc.gpsimd
                            q.dma_start(
                                o_d.ap()[
                                    tok0 + qt * 128 : tok0 + (qt + 1) * 128,
                                    eoh * 512 : (eoh + 1) * 512,
                                ],
                                o_sb[:],
                            )

    nc.compile()
    return nc


_NC_CACHE = {}


def _get_nc(nw=NW):
    if nw not in _NC_CACHE:
        _NC_CACHE[nw] = build_nc(nw)
    return _NC_CACHE[nw]


def _prep(x, Wq, bq, Wk, bk, Wv, bv, Wo, bo):
    """Host-side weight folding + per-core input maps."""
    x = np.asarray(x, dtype=np.float32)
    B, S, _ = x.shape
    Wq = np.asarray(Wq, np.float32)
    Wk = np.asarray(Wk, np.float32)
    Wv = np.asarray(Wv, np.float32)
    Wo = np.asarray(Wo, np.float32)
    bq = np.asarray(bq, np.float32)
    bv = np.asarray(bv, np.float32)
    bo = np.asarray(bo, np.float32)

    M = Wq.T @ Wk                      # scores = (X M) X^T  (+ col bias)
    Pt = (Wo @ Wv).T                   # Z = X @ Pt
    vvec = Wk.T @ bq                   # col bias, fused into y-projection
    bprime = Wo @ bv + bo              # folded into the residual below

    x_flat = x.reshape(B * S, E)
    x_resid = x_flat + bprime[None, :]
    t_core = B * S // N_CORES
    assert t_core == T

    common = {
        "wm": np.ascontiguousarray(M.astype(np.float16)),
        "wz": np.ascontiguousarray(Pt.astype(np.float16)),
        "vv": np.ascontiguousarray(vvec.reshape(ET, 128).T),
    }
    in_maps = [
        {
            "x": np.ascontiguousarray(
                x_resid[i * t_core : (i + 1) * t_core].astype(np.float16)
            ),
            "xt": np.ascontiguousarray(x_flat[i * t_core : (i + 1) * t_core].T.astype(np.float16)),
            **common,
        }
        for i in range(N_CORES)
    ]
    return in_maps


def kernel(x, Wq, bq, Wk, bk, Wv, bv, Wo, bo):
    in_maps = _prep(x, Wq, bq, Wk, bk, Wv, bv, Wo, bo)
    B, S = np.asarray(x).shape[:2]
    nc = _get_nc()
    res = run_bass_kernel_spmd(nc, in_maps, core_ids=list(range(N_CORES)))
    out = np.concatenate([res.results[i]["o"] for i in range(N_CORES)], axis=0)
    return out.reshape(B, S, E).astype(np.float32)
